# revision 17
# baseline (speedup 1.0000x reference)
"""Distributed Trainium2 kernel for the ADD rotation loss.

Math: the reference computes mean_{b,n} || point[b,n] @ (R_pred[b] - R_gt[b]) ||
with R_pred/R_gt rotation matrices. Because both are rotations,

    || p @ (Rp - Rg) || = 2 * | p x qv |,

where qv is the vector part of the relative quaternion q_pred * conj(q_gt).
With {E1, E2} an orthogonal basis of the plane perpendicular to qv, each
scaled to length |qv|,

    | p x qv |^2 = (p . E1)^2 + (p . E2)^2.

E1 is chosen as x_hat x qv (normalized), so E1.x == 0 identically: the E1
projection only needs the (y, z) point planes -> one fp8 DoubleRow matmul.
E2 takes DoubleRow (x, y) + a single-row z accumulate. 3 matmul passes per
(group, half) instead of 4.

The per-row coefficients (5 floats per batch row) are computed on the host
in float64, exactly mirroring the reference euler->rotation convention; the
device kernel is a pure streaming pipeline over the fp8 point tensor:

  per core (data-parallel over batch), per 128-row group:
    - one HWDGE DMA brings the group chunk (diag-stationary weights +
      planar x|y|z points per row-half)
    - TensorE: v0 (1 DR matmul/half), v1 (DR + single accumulate)
    - ACT: sq1 = Square(v1) (PSUM->SBUF bf16)
    - DVE: tot = v0^2 + sq1 via a custom fused square-add op (single PSUM
      read); for the HEAVY groups DVE also does sq1 via the NR op so
      ACT and DVE stay balanced
    - ACT: Sqrt(scale=4) with accum_out -> per-group partial sums
  Final tiny reduction (8 cores x 128 x slots) happens on the host.
"""

import sys

for _p in ("/opt/trn_rl_repo", "/root/.axon_site/_ro/trn_rl_repo"):
    if _p not in sys.path:
        sys.path.append(_p)

import numpy as np
import ml_dtypes

import concourse.bacc as bacc
import concourse.tile as tile
from concourse import mybir
from concourse.bass_utils import run_bass_kernel_spmd
from concourse import dve_ops as _dve_ops
from concourse import dve_spec as _dve_spec
from concourse.dve_uop import DveOpSpec
from concourse.dve_ops import RECIPROCAL_APPROX_NR

NCORES = 8
B = 8192
N = 1024
BSH = B // NCORES          # batch rows per core
G = BSH // 128             # b-groups of 128 rows per core
H = N // 2                 # points per row-half
F32 = mybir.dt.float32
BF16 = mybir.dt.bfloat16
F8 = mybir.dt.float8e4
AF = mybir.ActivationFunctionType
PM = mybir.MatmulPerfMode
E4M3 = ml_dtypes.float8_e4m3

W_G = 5 * 128              # fp8 stationary elems per row-group chunk (640)
PT_H = 3 * H               # fp8 point elems per row-half (1536)
C_G = W_G + 2 * PT_H       # chunk elems per (partition, group) (3712)
HEAVY = (1, 3, 5)          # groups whose sq(v1) runs on DVE instead of ACT

_CACHE = {}


def _register_sq_add():
    """out = in0^2 + in1 in one DVE pass (single PSUM read + SBUF bf16 add)."""
    name = "SQ_ADD_ANT"
    for op in _dve_ops.OPS:
        if op.name == name:
            return op
    spec = _dve_spec.Spec(
        body=_dve_spec.sq(_dve_spec.Src0) + _dve_spec.Src1,
        reference=lambda in0, in1, s0, s1, imm2: in0 * in0 + in1,
    )
    row = _dve_ops._CUSTOM_DVE_ROW_BASE + len(_dve_ops.OPS)
    shas = {}
    for ver in ("v3", "v4"):
        t = DveOpSpec(name=name, opcode=row,
                      uops=_dve_spec.lower(spec, ver=ver),
                      rd1_en=_dve_spec._has_src1(spec))
        shas[ver] = t.sha(ver)
    op = _dve_ops.DveOp(name, spec, subdim=False, uops_sha=shas)
    _dve_ops.OPS.append(op)
    _dve_ops.CUSTOM_DVE_SPECS[name] = spec
    _dve_ops._SUB_OPCODE_FOR_NAME[name] = row
    return op


SQ_ADD = _register_sq_add()


def build_bass():
    nc = bacc.Bacc("TRN2", target_bir_lowering=False, debug=False,
                   num_devices=NCORES)
    ptw = nc.declare_dram_parameter("ptw", [128, G * C_G], F8, isOutput=False)
    out = nc.declare_dram_parameter("out", [128, 8], F32, isOutput=True)

    def dv(a, b):
        return ptw[:, a:b]

    with tile.TileContext(nc) as tc:
        with (
            tc.tile_pool(name="const", bufs=1) as cp,
            tc.tile_pool(name="data", bufs=1) as dp,
            tc.tile_pool(name="psa", bufs=2, space="PSUM") as pa,
            tc.tile_pool(name="psb", bufs=2, space="PSUM") as pb,
            tc.tile_pool(name="sq", bufs=3) as qp,
        ):
            # ---- persistent tiles ----
            acc = cp.tile([128, 8], F32, name="acc", tag="acc")
            wrm = cp.tile([128, 1], F32, name="wrm", tag="wrm")
            mo = cp.tile([128, 1024], BF16, name="mo", tag="mo")
            wrs = cp.tile([128, 1], F32, name="wrs", tag="wrs")
            cw = cp.tile([128, 512], BF16, name="cw", tag="cw")

            # ---- chunk tiles (one per DMA trigger => clean deps) ----
            # chunk layout per (partition, group): [W 640 | h0 1536 | h1 1536]
            C0a = dp.tile([128, W_G + PT_H], F8, name="C0a", tag="C0a")
            C0b = dp.tile([128, PT_H], F8, name="C0b", tag="C0b")
            Cg = [None] + [dp.tile([128, C_G], F8, name=f"C{g}", tag=f"C{g}")
                           for g in range(1, G)]

            # ---- DMA triggers: HWDGE (sync + scalar rings), ordered by
            # need time within each ring. g0 split so the first matmul
            # starts earliest. ----
            def chunk_dma(eng, g):
                eng.dma_start(out=Cg[g][:, :], in_=dv(g * C_G, (g + 1) * C_G))

            nc.sync.dma_start(out=C0a[:, :], in_=dv(0, W_G + PT_H))
            chunk_dma(nc.scalar, 1)
            nc.sync.dma_start(out=C0b[:, :], in_=dv(W_G + PT_H, C_G))
            # warm constants / sqrt table behind the first triggers
            nc.vector.memset(cw[:, :], 0.0)
            nc.vector.memset(wrm[:, :], 1.0)
            # -1 constant for the DVE NR square trick: (0 - (-1)*v)*v = v^2.
            nc.vector.memset(mo[:, :], -1.0)
            nc.scalar.activation(out=wrs[:, :], in_=wrm[:, :], func=AF.Sqrt)
            for g in range(2, G):
                eng = nc.sync if g % 2 == 0 else nc.scalar
                chunk_dma(eng, g)

            def views(g):
                # -> (wv [128,5,128], halves (h0, h1) each [128,3,H])
                if g == 0:
                    wv = C0a[:, 0:W_G].rearrange("p (c q) -> p c q", c=5)
                    ha = C0a[:, W_G:W_G + PT_H].rearrange(
                        "p (c n) -> p c n", c=3)
                    hb = C0b[:, :].rearrange("p (c n) -> p c n", c=3)
                else:
                    c = Cg[g]
                    wv = c[:, 0:W_G].rearrange("p (c q) -> p c q", c=5)
                    ha = c[:, W_G:W_G + PT_H].rearrange(
                        "p (c n) -> p c n", c=3)
                    hb = c[:, W_G + PT_H:C_G].rearrange(
                        "p (c n) -> p c n", c=3)
                return wv, (ha, hb)

            def emit_mm(pva, pvb, g, h):
                wv, hs = views(g)
                t = hs[h]
                # v0 = E1y*py + E1z*pz  (E1x == 0 by construction)
                nc.tensor.matmul(out=pva[:, h, :], lhsT=wv[:, 0:2, :],
                                 rhs=t[:, 1:3, :], start=True, stop=True,
                                 perf_mode=PM.DoubleRow)
                # v1 = E2x*px + E2y*py + E2z*pz
                nc.tensor.matmul(out=pvb[:, h, :], lhsT=wv[:, 2:4, :],
                                 rhs=t[:, 0:2, :], start=True, stop=False,
                                 perf_mode=PM.DoubleRow)
                nc.tensor.matmul(out=pvb[:, h, :], lhsT=wv[:, 4, :],
                                 rhs=t[:, 2, :], start=False, stop=True)

            # ---- PE p-state warm-up: many TINY junk matmuls (N=64) on a
            # zeroed tile while the first point chunk is in flight. The HAM
            # clock-gate needs ~3.4us of sustained PE activity to unthrottle;
            # fine-grained junk MMs cover that window yet release the PE
            # within ~60ns of the first real matmul's data arriving. ----
            wpv = pa.tile([128, 2, H], F32, name="pva", tag="pva")
            for k in range(56):
                nc.tensor.matmul(out=wpv[:, 0, 0:64], lhsT=cw[:, 0:128],
                                 rhs=cw[:, 0:64], start=True, stop=True)

            # ---- main loop ----
            # ACT: sq1 for normal groups + all sqrts; DVE: fused v0^2+sq1
            # (and sq1 via NR for HEAVY groups). Sqrt for pair p is emitted
            # one group late so the ACT FIFO never blocks a sq1.
            tots = {}
            sq7 = None

            def emit_sq1_act(g, pvb):
                sq1 = qp.tile([128, 1024], BF16, name="sq1", tag="sq1")
                nc.scalar.activation(
                    out=sq1[:, :],
                    in_=pvb[:, :, :].rearrange("p h n -> p (h n)"),
                    func=AF.Square)
                return sq1

            def emit_sq1_dve(g, pvb):
                sq1 = qp.tile([128, 1024], BF16, name="sq1", tag="sq1")
                nc.vector._custom_dve(
                    RECIPROCAL_APPROX_NR, out=sq1[:, :], in0=mo[:, :],
                    in1=pvb[:, :, :].rearrange("p h n -> p (h n)"), s0=0.0)
                return sq1

            def emit_fused(g, pva, sq1):
                if g % 2 == 0:
                    tots[g // 2] = qp.tile([128, 2, 1024], BF16,
                                           name="tot", tag="tot")
                tot = tots[g // 2]
                nc.vector._custom_dve(
                    SQ_ADD, out=tot[:, g % 2, :],
                    in0=pva[:, :, :].rearrange("p h n -> p (h n)"),
                    in1=sq1[:, :])

            def emit_sqrt(pair, slot, width=2048):
                dist = qp.tile([128, 2048], BF16, name="dist", tag="dist")
                nc.scalar.activation(
                    out=dist[:, 0:width],
                    in_=tots[pair][:, :, :].rearrange(
                        "p a b -> p (a b)")[:, 0:width],
                    func=AF.Sqrt, scale=4.0,
                    accum_out=acc[:, slot:slot + 1])

            for g in range(7):
                pva = pa.tile([128, 2, H], F32, name="pva", tag="pva")
                pvb = pb.tile([128, 2, H], F32, name="pvb", tag="pvb")
                emit_mm(pva, pvb, g, 0)
                emit_mm(pva, pvb, g, 1)
                if g in HEAVY:
                    sq1 = emit_sq1_dve(g, pvb)
                else:
                    sq1 = emit_sq1_act(g, pvb)
                # sqrt for pair p emitted two groups late so the ACT FIFO
                # never blocks a square behind a not-yet-ready tot
                if g == 2:
                    emit_sqrt(0, slot=0)
                elif g == 4:
                    emit_sqrt(1, slot=1)
                elif g == 6:
                    emit_sqrt(2, slot=2)
                emit_fused(g, pva, sq1)
                if g == 4:
                    nc.sync.dma_start(out=out[:, 0:2], in_=acc[:, 0:2])

            # group 7, per half-row-block to shorten the serial tail
            pva = pa.tile([128, 2, H], F32, name="pva", tag="pva")
            pvb = pb.tile([128, 2, H], F32, name="pvb", tag="pvb")
            sq7 = qp.tile([128, 2, H], BF16, name="sq7", tag="sq7")
            tot7 = qp.tile([128, 2, H], BF16, name="tot7", tag="tot7")
            dist7 = qp.tile([128, 2, H], BF16, name="dist7", tag="dist7")
            dist6 = qp.tile([128, 1024], BF16, name="dist6", tag="dist6")
            for h in (0, 1):
                emit_mm(pva, pvb, 7, h)
                nc.scalar.activation(out=sq7[:, h, :], in_=pvb[:, h, :],
                                     func=AF.Square)
                if h == 0:
                    # sqrt for group 6 rides the h1-matmul window
                    nc.scalar.activation(
                        out=dist6[:, :],
                        in_=tots[3][:, 0, :],
                        func=AF.Sqrt, scale=4.0, accum_out=acc[:, 3:4])
                nc.vector._custom_dve(
                    SQ_ADD, out=tot7[:, h, :], in0=pva[:, h, :],
                    in1=sq7[:, h, :])
            nc.scalar.activation(
                out=dist7[:, :, :].rearrange("p a b -> p (a b)"),
                in_=tot7[:, :, :].rearrange("p a b -> p (a b)"),
                func=AF.Sqrt, scale=4.0, accum_out=acc[:, 4:5])
            nc.sync.dma_start(out=out[:, 2:5], in_=acc[:, 2:5])

    nc.finalize()
    return nc


# ---------------- host-side coefficient math ----------------

def _host_ew(pred, mode, gt):
    """E1/E2 per batch row, float64, mirroring the reference math."""
    p = pred.astype(np.float64)
    md = mode.astype(np.float64)
    m1, m2, m3, m4 = p[:, 0], p[:, 1], p[:, 2], p[:, 3]
    sgn = np.where(md > 0.5, 1.0, -1.0)
    e2 = sgn * np.arcsin(np.sqrt(m3 ** 2 / (m1 ** 2 + m2 ** 2 + m3 ** 2)))
    e3 = np.arctan2(m4, m3 / (np.sin(e2) + 1e-9))
    tmp = np.cos(e2) * np.cos(e3)
    e1 = np.arctan2(m2 / tmp, m1 / tmp)
    e3 = np.where(e3 > 0, e3, e3 + 2 * np.pi)
    ep = np.stack([e1, e2, e3], -1)
    eg = gt.astype(np.float64)

    def quat_xyz(e):
        # q = qx(a) * qy(b) * qz(c) for R = Rx(a) Ry(b) Rz(c)
        a, b, c = e[:, 0] / 2, e[:, 1] / 2, e[:, 2] / 2
        ca, sa = np.cos(a), np.sin(a)
        cb, sb = np.cos(b), np.sin(b)
        cc, sc = np.cos(c), np.sin(c)
        w = ca * cb * cc - sa * sb * sc
        x = sa * cb * cc + ca * sb * sc
        y = ca * sb * cc - sa * cb * sc
        z = ca * cb * sc + sa * sb * cc
        return w, np.stack([x, y, z], -1)

    wp, vp = quat_xyz(ep)
    wg, vg = quat_xyz(eg)
    qv = wg[:, None] * vp - wp[:, None] * vg - np.cross(vp, vg)

    qx, qy, qz = qv[:, 0], qv[:, 1], qv[:, 2]
    s = qy ** 2 + qz ** 2
    n = np.sqrt(s + qx ** 2)
    r = 1.0 / np.sqrt(s + 1e-250)
    t1 = n * r
    # E1 = (0, qz, -qy) * t1  (x component identically zero -> dropped)
    # E2 = (-s, qx*qy, qx*qz) * r
    return np.stack([qz * t1, -qy * t1,
                     -s * r, qx * qy * r, qx * qz * r], -1)   # [B, 5]


def _pack_inputs(pred, mode, gt, point):
    w5 = _host_ew(np.asarray(pred), np.asarray(mode), np.asarray(gt))
    w5q = w5.astype(np.float32).astype(E4M3)           # [B, 5]
    ptq = np.asarray(point, dtype=np.float32).astype(E4M3)  # [B, N, 3]

    in_maps = []
    idx = np.arange(128)
    for c in range(NCORES):
        sl = slice(c * BSH, (c + 1) * BSH)
        # row b_local = p*G + g
        w5c = w5q[sl].reshape(128, G, 5)
        wtc = np.zeros((128, G, 5, 128), dtype=E4M3)
        wtc[idx, :, :, idx] = w5c
        ptc = (ptq[sl].reshape(128, G, 2, H, 3)
               .transpose(0, 1, 2, 4, 3))              # [p, g, h, c, n]
        chunk = np.concatenate(
            [wtc.reshape(128, G, W_G), ptc.reshape(128, G, 2 * PT_H)], axis=2)
        in_maps.append({"ptw": np.ascontiguousarray(chunk)
                        .reshape(128, G * C_G)})
    return in_maps


def _get_nc():
    if "nc" not in _CACHE:
        _CACHE["nc"] = build_bass()
    return _CACHE["nc"]


def kernel(pred, mode, gt, point, **run_kwargs):
    nc = _get_nc()
    in_maps = _pack_inputs(pred, mode, gt, point)
    res = run_bass_kernel_spmd(nc, in_maps, core_ids=list(range(NCORES)),
                               **run_kwargs)
    total = sum(float(r["out"][:, 0:5].astype(np.float64).sum())
                for r in res.results)
    result = np.float32(total / (B * N))
    if run_kwargs:
        return result, res
    return result


# revision 18
# speedup vs baseline: 1.0809x; 1.0809x over previous
"""Distributed Trainium2 kernel for the ADD rotation loss.

Math: the reference computes mean_{b,n} || point[b,n] @ (R_pred[b] - R_gt[b]) ||
with R_pred/R_gt rotation matrices. Because both are rotations,

    || p @ (Rp - Rg) || = 2 * | p x qv |,

where qv is the vector part of the relative quaternion q_pred * conj(q_gt).
With {E1, E2} an orthogonal basis of the plane perpendicular to qv, each
scaled to length |qv|,

    | p x qv |^2 = (p . E1)^2 + (p . E2)^2.

E1 is chosen as x_hat x qv (normalized), so E1.x == 0 identically: the E1
projection only needs the (y, z) point planes -> one fp8 DoubleRow matmul.
E2 takes DoubleRow (x, y) + a single-row z accumulate. 3 matmul passes per
(group, half) instead of 4.

The per-row coefficients (5 floats per batch row) are computed on the host
in float64, exactly mirroring the reference euler->rotation convention; the
device kernel is a pure streaming pipeline over the fp8 point tensor:

  per core (data-parallel over batch), per 128-row group:
    - one HWDGE DMA brings the group chunk (diag-stationary weights +
      planar x|y|z points per row-half)
    - TensorE: v0 (1 DR matmul/half), v1 (DR + single accumulate)
    - ACT: sq1 = Square(v1) (PSUM->SBUF bf16)
    - DVE: tot = v0^2 + sq1 via a custom fused square-add op (single PSUM
      read); for the HEAVY groups DVE also does sq1 via the NR op so
      ACT and DVE stay balanced
    - ACT: Sqrt(scale=4) with accum_out -> per-group partial sums
  Final tiny reduction (8 cores x 128 x slots) happens on the host.
"""

import sys

for _p in ("/opt/trn_rl_repo", "/root/.axon_site/_ro/trn_rl_repo"):
    if _p not in sys.path:
        sys.path.append(_p)

import numpy as np
import ml_dtypes

import concourse.bacc as bacc
import concourse.tile as tile
from concourse import mybir
from concourse.bass_utils import run_bass_kernel_spmd
from concourse import dve_ops as _dve_ops
from concourse import dve_spec as _dve_spec
from concourse.dve_uop import DveOpSpec
from concourse.dve_ops import RECIPROCAL_APPROX_NR

NCORES = 8
B = 8192
N = 1024
BSH = B // NCORES          # batch rows per core
G = BSH // 128             # b-groups of 128 rows per core
H = N // 2                 # points per row-half
F32 = mybir.dt.float32
BF16 = mybir.dt.bfloat16
F8 = mybir.dt.float8e4
AF = mybir.ActivationFunctionType
PM = mybir.MatmulPerfMode
E4M3 = ml_dtypes.float8_e4m3

W_G = 5 * 128              # fp8 stationary elems per row-group chunk (640)
PT_H = 3 * H               # fp8 point elems per row-half (1536)
C_G = W_G + 2 * PT_H       # chunk elems per (partition, group) (3712)
HEAVY = (1, 3, 5)          # groups whose sq(v1) runs on DVE instead of ACT

_CACHE = {}


def _register_sq_add():
    """out = in0^2 + in1 in one DVE pass (single PSUM read + SBUF bf16 add)."""
    name = "SQ_ADD_ANT"
    for op in _dve_ops.OPS:
        if op.name == name:
            return op
    spec = _dve_spec.Spec(
        body=_dve_spec.sq(_dve_spec.Src0) + _dve_spec.Src1,
        reference=lambda in0, in1, s0, s1, imm2: in0 * in0 + in1,
    )
    row = _dve_ops._CUSTOM_DVE_ROW_BASE + len(_dve_ops.OPS)
    shas = {}
    for ver in ("v3", "v4"):
        t = DveOpSpec(name=name, opcode=row,
                      uops=_dve_spec.lower(spec, ver=ver),
                      rd1_en=_dve_spec._has_src1(spec))
        shas[ver] = t.sha(ver)
    op = _dve_ops.DveOp(name, spec, subdim=False, uops_sha=shas)
    _dve_ops.OPS.append(op)
    _dve_ops.CUSTOM_DVE_SPECS[name] = spec
    _dve_ops._SUB_OPCODE_FOR_NAME[name] = row
    return op


SQ_ADD = _register_sq_add()


def _install_lean_exit():
    """Trim the TileContext exit sequence.

    The stock `_drain_and_barrier` emits a sync drain gated on the global
    tile clock, then TWO all-engine butterfly barriers around a semaphore
    range-clear — ~290 sequencer instructions, ~7us of measured postamble.
    The drain (which waits for every tracked completion, including the
    final output DMA) is all that correctness of a single execution needs;
    engine preambles re-initialize semaphore/event state on the next load.
    """
    from concourse.vector_clock import ScopedClock

    def _lean(self, tick_clock, wait_clock):
        drain_inst = self.nc.sync.drain()
        wait_clock.add_sem_waits(
            drain_inst.ins, ScopedClock({None: tick_clock.global_clock})
        )
        popped = self.nc._tile_sem_poison_stack.pop()
        assert popped is self._sem_poison

    tile.TileContext._drain_and_barrier = _lean


_install_lean_exit()


def build_bass():
    nc = bacc.Bacc("TRN2", target_bir_lowering=False, debug=False,
                   num_devices=NCORES)
    ptw = nc.declare_dram_parameter("ptw", [128, G * C_G], F8, isOutput=False)
    out = nc.declare_dram_parameter("out", [128, 8], F32, isOutput=True)

    def dv(a, b):
        return ptw[:, a:b]

    with tile.TileContext(nc) as tc:
        with (
            tc.tile_pool(name="const", bufs=1) as cp,
            tc.tile_pool(name="data", bufs=1) as dp,
            tc.tile_pool(name="psa", bufs=2, space="PSUM") as pa,
            tc.tile_pool(name="psb", bufs=2, space="PSUM") as pb,
            tc.tile_pool(name="sq", bufs=3) as qp,
        ):
            # ---- persistent tiles ----
            acc = cp.tile([128, 8], F32, name="acc", tag="acc")
            wrm = cp.tile([128, 1], F32, name="wrm", tag="wrm")
            mo = cp.tile([128, 1024], BF16, name="mo", tag="mo")
            wrs = cp.tile([128, 1], F32, name="wrs", tag="wrs")
            cw = cp.tile([128, 512], BF16, name="cw", tag="cw")

            # ---- chunk tiles (one per DMA trigger => clean deps) ----
            # chunk layout per (partition, group): [W 640 | h0 1536 | h1 1536]
            C0a = dp.tile([128, W_G + PT_H], F8, name="C0a", tag="C0a")
            C0b = dp.tile([128, PT_H], F8, name="C0b", tag="C0b")
            Cg = [None] + [dp.tile([128, C_G], F8, name=f"C{g}", tag=f"C{g}")
                           for g in range(1, G)]

            # ---- DMA triggers: HWDGE (sync + scalar rings), ordered by
            # need time within each ring. g0 split so the first matmul
            # starts earliest. ----
            def chunk_dma(eng, g):
                eng.dma_start(out=Cg[g][:, :], in_=dv(g * C_G, (g + 1) * C_G))

            nc.sync.dma_start(out=C0a[:, :], in_=dv(0, W_G + PT_H))
            chunk_dma(nc.scalar, 1)
            nc.sync.dma_start(out=C0b[:, :], in_=dv(W_G + PT_H, C_G))
            # warm constants / sqrt table behind the first triggers
            nc.vector.memset(cw[:, :], 0.0)
            nc.vector.memset(wrm[:, :], 1.0)
            # -1 constant for the DVE NR square trick: (0 - (-1)*v)*v = v^2.
            nc.vector.memset(mo[:, :], -1.0)
            nc.scalar.activation(out=wrs[:, :], in_=wrm[:, :], func=AF.Sqrt)
            for g in range(2, G):
                eng = nc.sync if g % 2 == 0 else nc.scalar
                chunk_dma(eng, g)

            def views(g):
                # -> (wv [128,5,128], halves (h0, h1) each [128,3,H])
                if g == 0:
                    wv = C0a[:, 0:W_G].rearrange("p (c q) -> p c q", c=5)
                    ha = C0a[:, W_G:W_G + PT_H].rearrange(
                        "p (c n) -> p c n", c=3)
                    hb = C0b[:, :].rearrange("p (c n) -> p c n", c=3)
                else:
                    c = Cg[g]
                    wv = c[:, 0:W_G].rearrange("p (c q) -> p c q", c=5)
                    ha = c[:, W_G:W_G + PT_H].rearrange(
                        "p (c n) -> p c n", c=3)
                    hb = c[:, W_G + PT_H:C_G].rearrange(
                        "p (c n) -> p c n", c=3)
                return wv, (ha, hb)

            def emit_mm(pva, pvb, g, h):
                wv, hs = views(g)
                t = hs[h]
                # v0 = E1y*py + E1z*pz  (E1x == 0 by construction)
                nc.tensor.matmul(out=pva[:, h, :], lhsT=wv[:, 0:2, :],
                                 rhs=t[:, 1:3, :], start=True, stop=True,
                                 perf_mode=PM.DoubleRow)
                # v1 = E2x*px + E2y*py + E2z*pz
                nc.tensor.matmul(out=pvb[:, h, :], lhsT=wv[:, 2:4, :],
                                 rhs=t[:, 0:2, :], start=True, stop=False,
                                 perf_mode=PM.DoubleRow)
                nc.tensor.matmul(out=pvb[:, h, :], lhsT=wv[:, 4, :],
                                 rhs=t[:, 2, :], start=False, stop=True)

            # ---- PE p-state warm-up: many TINY junk matmuls (N=64) on a
            # zeroed tile while the first point chunk is in flight. The HAM
            # clock-gate needs ~3.4us of sustained PE activity to unthrottle;
            # fine-grained junk MMs cover that window yet release the PE
            # within ~60ns of the first real matmul's data arriving. ----
            wpv = pa.tile([128, 2, H], F32, name="pva", tag="pva")
            for k in range(56):
                nc.tensor.matmul(out=wpv[:, 0, 0:64], lhsT=cw[:, 0:128],
                                 rhs=cw[:, 0:64], start=True, stop=True)

            # ---- main loop ----
            # ACT: sq1 for normal groups + all sqrts; DVE: fused v0^2+sq1
            # (and sq1 via NR for HEAVY groups). Sqrt for pair p is emitted
            # one group late so the ACT FIFO never blocks a sq1.
            tots = {}
            sq7 = None

            def emit_sq1_act(g, pvb):
                sq1 = qp.tile([128, 1024], BF16, name="sq1", tag="sq1")
                nc.scalar.activation(
                    out=sq1[:, :],
                    in_=pvb[:, :, :].rearrange("p h n -> p (h n)"),
                    func=AF.Square)
                return sq1

            def emit_sq1_dve(g, pvb):
                sq1 = qp.tile([128, 1024], BF16, name="sq1", tag="sq1")
                nc.vector._custom_dve(
                    RECIPROCAL_APPROX_NR, out=sq1[:, :], in0=mo[:, :],
                    in1=pvb[:, :, :].rearrange("p h n -> p (h n)"), s0=0.0)
                return sq1

            def emit_fused(g, pva, sq1):
                if g % 2 == 0:
                    tots[g // 2] = qp.tile([128, 2, 1024], BF16,
                                           name="tot", tag="tot")
                tot = tots[g // 2]
                nc.vector._custom_dve(
                    SQ_ADD, out=tot[:, g % 2, :],
                    in0=pva[:, :, :].rearrange("p h n -> p (h n)"),
                    in1=sq1[:, :])

            def emit_sqrt(pair, slot, width=2048):
                dist = qp.tile([128, 2048], BF16, name="dist", tag="dist")
                nc.scalar.activation(
                    out=dist[:, 0:width],
                    in_=tots[pair][:, :, :].rearrange(
                        "p a b -> p (a b)")[:, 0:width],
                    func=AF.Sqrt, scale=4.0,
                    accum_out=acc[:, slot:slot + 1])

            for g in range(7):
                pva = pa.tile([128, 2, H], F32, name="pva", tag="pva")
                pvb = pb.tile([128, 2, H], F32, name="pvb", tag="pvb")
                emit_mm(pva, pvb, g, 0)
                emit_mm(pva, pvb, g, 1)
                if g in HEAVY:
                    sq1 = emit_sq1_dve(g, pvb)
                else:
                    sq1 = emit_sq1_act(g, pvb)
                # sqrt for pair p emitted two groups late so the ACT FIFO
                # never blocks a square behind a not-yet-ready tot
                if g == 2:
                    emit_sqrt(0, slot=0)
                elif g == 4:
                    emit_sqrt(1, slot=1)
                elif g == 6:
                    emit_sqrt(2, slot=2)
                emit_fused(g, pva, sq1)
                if g == 4:
                    nc.sync.dma_start(out=out[:, 0:2], in_=acc[:, 0:2])

            # group 7, per half-row-block to shorten the serial tail
            pva = pa.tile([128, 2, H], F32, name="pva", tag="pva")
            pvb = pb.tile([128, 2, H], F32, name="pvb", tag="pvb")
            sq7 = qp.tile([128, 2, H], BF16, name="sq7", tag="sq7")
            tot7 = qp.tile([128, 2, H], BF16, name="tot7", tag="tot7")
            dist7 = qp.tile([128, 2, H], BF16, name="dist7", tag="dist7")
            dist6 = qp.tile([128, 1024], BF16, name="dist6", tag="dist6")
            for h in (0, 1):
                emit_mm(pva, pvb, 7, h)
                nc.scalar.activation(out=sq7[:, h, :], in_=pvb[:, h, :],
                                     func=AF.Square)
                if h == 0:
                    # sqrt for group 6 rides the h1-matmul window
                    nc.scalar.activation(
                        out=dist6[:, :],
                        in_=tots[3][:, 0, :],
                        func=AF.Sqrt, scale=4.0, accum_out=acc[:, 3:4])
                nc.vector._custom_dve(
                    SQ_ADD, out=tot7[:, h, :], in0=pva[:, h, :],
                    in1=sq7[:, h, :])
            nc.scalar.activation(
                out=dist7[:, :, :].rearrange("p a b -> p (a b)"),
                in_=tot7[:, :, :].rearrange("p a b -> p (a b)"),
                func=AF.Sqrt, scale=4.0, accum_out=acc[:, 4:5])
            nc.sync.dma_start(out=out[:, 2:5], in_=acc[:, 2:5])

    nc.finalize()
    return nc


# ---------------- host-side coefficient math ----------------

def _host_ew(pred, mode, gt):
    """E1/E2 per batch row, float64, mirroring the reference math."""
    p = pred.astype(np.float64)
    md = mode.astype(np.float64)
    m1, m2, m3, m4 = p[:, 0], p[:, 1], p[:, 2], p[:, 3]
    sgn = np.where(md > 0.5, 1.0, -1.0)
    e2 = sgn * np.arcsin(np.sqrt(m3 ** 2 / (m1 ** 2 + m2 ** 2 + m3 ** 2)))
    e3 = np.arctan2(m4, m3 / (np.sin(e2) + 1e-9))
    tmp = np.cos(e2) * np.cos(e3)
    e1 = np.arctan2(m2 / tmp, m1 / tmp)
    e3 = np.where(e3 > 0, e3, e3 + 2 * np.pi)
    ep = np.stack([e1, e2, e3], -1)
    eg = gt.astype(np.float64)

    def quat_xyz(e):
        # q = qx(a) * qy(b) * qz(c) for R = Rx(a) Ry(b) Rz(c)
        a, b, c = e[:, 0] / 2, e[:, 1] / 2, e[:, 2] / 2
        ca, sa = np.cos(a), np.sin(a)
        cb, sb = np.cos(b), np.sin(b)
        cc, sc = np.cos(c), np.sin(c)
        w = ca * cb * cc - sa * sb * sc
        x = sa * cb * cc + ca * sb * sc
        y = ca * sb * cc - sa * cb * sc
        z = ca * cb * sc + sa * sb * cc
        return w, np.stack([x, y, z], -1)

    wp, vp = quat_xyz(ep)
    wg, vg = quat_xyz(eg)
    qv = wg[:, None] * vp - wp[:, None] * vg - np.cross(vp, vg)

    qx, qy, qz = qv[:, 0], qv[:, 1], qv[:, 2]
    s = qy ** 2 + qz ** 2
    n = np.sqrt(s + qx ** 2)
    r = 1.0 / np.sqrt(s + 1e-250)
    t1 = n * r
    # E1 = (0, qz, -qy) * t1  (x component identically zero -> dropped)
    # E2 = (-s, qx*qy, qx*qz) * r
    return np.stack([qz * t1, -qy * t1,
                     -s * r, qx * qy * r, qx * qz * r], -1)   # [B, 5]


def _pack_inputs(pred, mode, gt, point):
    w5 = _host_ew(np.asarray(pred), np.asarray(mode), np.asarray(gt))
    w5q = w5.astype(np.float32).astype(E4M3)           # [B, 5]
    ptq = np.asarray(point, dtype=np.float32).astype(E4M3)  # [B, N, 3]

    in_maps = []
    idx = np.arange(128)
    for c in range(NCORES):
        sl = slice(c * BSH, (c + 1) * BSH)
        # row b_local = p*G + g
        w5c = w5q[sl].reshape(128, G, 5)
        wtc = np.zeros((128, G, 5, 128), dtype=E4M3)
        wtc[idx, :, :, idx] = w5c
        ptc = (ptq[sl].reshape(128, G, 2, H, 3)
               .transpose(0, 1, 2, 4, 3))              # [p, g, h, c, n]
        chunk = np.concatenate(
            [wtc.reshape(128, G, W_G), ptc.reshape(128, G, 2 * PT_H)], axis=2)
        in_maps.append({"ptw": np.ascontiguousarray(chunk)
                        .reshape(128, G * C_G)})
    return in_maps


def _get_nc():
    if "nc" not in _CACHE:
        _CACHE["nc"] = build_bass()
    return _CACHE["nc"]


def kernel(pred, mode, gt, point, **run_kwargs):
    nc = _get_nc()
    in_maps = _pack_inputs(pred, mode, gt, point)
    res = run_bass_kernel_spmd(nc, in_maps, core_ids=list(range(NCORES)),
                               **run_kwargs)
    total = sum(float(r["out"][:, 0:5].astype(np.float64).sum())
                for r in res.results)
    result = np.float32(total / (B * N))
    if run_kwargs:
        return result, res
    return result


# revision 21
# speedup vs baseline: 1.1055x; 1.0228x over previous
"""Distributed Trainium2 kernel for the ADD rotation loss.

Math: the reference computes mean_{b,n} || point[b,n] @ (R_pred[b] - R_gt[b]) ||
with R_pred/R_gt rotation matrices. Because both are rotations,

    || p @ (Rp - Rg) || = 2 * | p x qv |,

where qv is the vector part of the relative quaternion q_pred * conj(q_gt).
With {E1, E2} an orthogonal basis of the plane perpendicular to qv, each
scaled to length |qv|,

    | p x qv |^2 = (p . E1)^2 + (p . E2)^2.

E1 is chosen as x_hat x qv (normalized), so E1.x == 0 identically: the E1
projection only needs the (y, z) point planes -> one fp8 DoubleRow matmul.
E2 takes DoubleRow (x, y) + a single-row z accumulate. 3 matmul passes per
(group, half) instead of 4.

The per-row coefficients (5 floats per batch row) are computed on the host
in float64, exactly mirroring the reference euler->rotation convention; the
device kernel is a pure streaming pipeline over the fp8 point tensor:

  per core (data-parallel over batch), per 128-row group:
    - one HWDGE DMA brings the group chunk (diag-stationary weights +
      planar x|y|z points per row-half)
    - TensorE: v0 (1 DR matmul/half), v1 (DR + single accumulate)
    - ACT: sq1 = Square(v1) (PSUM->SBUF bf16)
    - DVE: tot = v0^2 + sq1 via a custom fused square-add op (single PSUM
      read); for the HEAVY groups DVE also does sq1 via the NR op so
      ACT and DVE stay balanced
    - ACT: Sqrt(scale=4) with accum_out -> per-group partial sums
  Final tiny reduction (8 cores x 128 x slots) happens on the host.
"""

import sys

for _p in ("/opt/trn_rl_repo", "/root/.axon_site/_ro/trn_rl_repo"):
    if _p not in sys.path:
        sys.path.append(_p)

import numpy as np
import ml_dtypes

import concourse.bacc as bacc
import concourse.tile as tile
from concourse import mybir
from concourse.bass_utils import run_bass_kernel_spmd
from concourse import dve_ops as _dve_ops
from concourse import dve_spec as _dve_spec
from concourse.dve_uop import DveOpSpec
from concourse.dve_ops import RECIPROCAL_APPROX_NR

NCORES = 8
B = 8192
N = 1024
BSH = B // NCORES          # batch rows per core
G = BSH // 128             # b-groups of 128 rows per core
H = N // 2                 # points per row-half
F32 = mybir.dt.float32
BF16 = mybir.dt.bfloat16
F8 = mybir.dt.float8e4
AF = mybir.ActivationFunctionType
PM = mybir.MatmulPerfMode
E4M3 = ml_dtypes.float8_e4m3

W_G = 5 * 128              # fp8 stationary elems per row-group chunk (640)
PT_H = 3 * H               # fp8 point elems per row-half (1536)
C_G = W_G + 2 * PT_H       # chunk elems per (partition, group) (3712)
HEAVY = (1, 3, 5)          # groups whose sq(v1) runs on DVE instead of ACT

_CACHE = {}


def _register_sq_add():
    """out = in0^2 + in1 in one DVE pass (single PSUM read + SBUF bf16 add)."""
    name = "SQ_ADD_ANT"
    for op in _dve_ops.OPS:
        if op.name == name:
            return op
    spec = _dve_spec.Spec(
        body=_dve_spec.sq(_dve_spec.Src0) + _dve_spec.Src1,
        reference=lambda in0, in1, s0, s1, imm2: in0 * in0 + in1,
    )
    row = _dve_ops._CUSTOM_DVE_ROW_BASE + len(_dve_ops.OPS)
    shas = {}
    for ver in ("v3", "v4"):
        t = DveOpSpec(name=name, opcode=row,
                      uops=_dve_spec.lower(spec, ver=ver),
                      rd1_en=_dve_spec._has_src1(spec))
        shas[ver] = t.sha(ver)
    op = _dve_ops.DveOp(name, spec, subdim=False, uops_sha=shas)
    _dve_ops.OPS.append(op)
    _dve_ops.CUSTOM_DVE_SPECS[name] = spec
    _dve_ops._SUB_OPCODE_FOR_NAME[name] = row
    return op


SQ_ADD = _register_sq_add()


def _install_lean_exit():
    """Trim the TileContext exit sequence.

    The stock `_drain_and_barrier` emits a sync drain gated on the global
    tile clock, then TWO all-engine butterfly barriers around a semaphore
    range-clear — ~290 sequencer instructions, ~7us of measured postamble.
    The drain (which waits for every tracked completion, including the
    final output DMA) is all that correctness of a single execution needs;
    engine preambles re-initialize semaphore/event state on the next load.
    """
    from concourse.vector_clock import ScopedClock

    def _lean(self, tick_clock, wait_clock):
        drain_inst = self.nc.sync.drain()
        wait_clock.add_sem_waits(
            drain_inst.ins, ScopedClock({None: tick_clock.global_clock})
        )
        popped = self.nc._tile_sem_poison_stack.pop()
        assert popped is self._sem_poison

    tile.TileContext._drain_and_barrier = _lean


_install_lean_exit()


def build_bass():
    nc = bacc.Bacc("TRN2", target_bir_lowering=False, debug=False,
                   num_devices=NCORES)
    ptw = nc.declare_dram_parameter("ptw", [128, G * C_G], F8, isOutput=False)
    out = nc.declare_dram_parameter("out", [128, 8], F32, isOutput=True)

    def dv(a, b):
        return ptw[:, a:b]

    with tile.TileContext(nc) as tc:
        with (
            tc.tile_pool(name="const", bufs=1) as cp,
            tc.tile_pool(name="data", bufs=1) as dp,
            tc.tile_pool(name="psa", bufs=2, space="PSUM") as pa,
            tc.tile_pool(name="psb", bufs=2, space="PSUM") as pb,
            tc.tile_pool(name="sq", bufs=3) as qp,
        ):
            # ---- persistent tiles ----
            acc = cp.tile([128, 8], F32, name="acc", tag="acc")
            wrm = cp.tile([128, 1], F32, name="wrm", tag="wrm")
            mo = cp.tile([128, 1024], BF16, name="mo", tag="mo")
            wrs = cp.tile([128, 1], F32, name="wrs", tag="wrs")
            cw = cp.tile([128, 512], BF16, name="cw", tag="cw")

            # ---- chunk tiles (one per DMA trigger => clean deps) ----
            # chunk layout per (partition, group): [W 640 | h0 1536 | h1 1536]
            C0a = dp.tile([128, W_G + PT_H], F8, name="C0a", tag="C0a")
            C0b = dp.tile([128, PT_H], F8, name="C0b", tag="C0b")
            Cg = [None] + [dp.tile([128, C_G], F8, name=f"C{g}", tag=f"C{g}")
                           for g in range(1, G)]

            # ---- DMA triggers: HWDGE (sync + scalar rings), ordered by
            # need time within each ring. g0 split so the first matmul
            # starts earliest. ----
            def chunk_dma(eng, g):
                eng.dma_start(out=Cg[g][:, :], in_=dv(g * C_G, (g + 1) * C_G))

            nc.sync.dma_start(out=C0a[:, :], in_=dv(0, W_G + PT_H))
            chunk_dma(nc.scalar, 1)
            nc.sync.dma_start(out=C0b[:, :], in_=dv(W_G + PT_H, C_G))
            # warm constants / sqrt table behind the first triggers
            nc.vector.memset(cw[:, :], 0.0)
            nc.vector.memset(wrm[:, :], 1.0)
            # -1 constant for the DVE NR square trick: (0 - (-1)*v)*v = v^2.
            nc.vector.memset(mo[:, :], -1.0)
            nc.scalar.activation(out=wrs[:, :], in_=wrm[:, :], func=AF.Sqrt)
            for g in range(2, G):
                eng = nc.sync if g % 2 == 0 else nc.scalar
                chunk_dma(eng, g)

            def views(g):
                # -> (wv [128,5,128], halves (h0, h1) each [128,3,H])
                if g == 0:
                    wv = C0a[:, 0:W_G].rearrange("p (c q) -> p c q", c=5)
                    ha = C0a[:, W_G:W_G + PT_H].rearrange(
                        "p (c n) -> p c n", c=3)
                    hb = C0b[:, :].rearrange("p (c n) -> p c n", c=3)
                else:
                    c = Cg[g]
                    wv = c[:, 0:W_G].rearrange("p (c q) -> p c q", c=5)
                    ha = c[:, W_G:W_G + PT_H].rearrange(
                        "p (c n) -> p c n", c=3)
                    hb = c[:, W_G + PT_H:C_G].rearrange(
                        "p (c n) -> p c n", c=3)
                return wv, (ha, hb)

            def emit_mm(pva, pvb, g, h):
                wv, hs = views(g)
                t = hs[h]
                # v0 = E1y*py + E1z*pz  (E1x == 0 by construction)
                nc.tensor.matmul(out=pva[:, h, :], lhsT=wv[:, 0:2, :],
                                 rhs=t[:, 1:3, :], start=True, stop=True,
                                 perf_mode=PM.DoubleRow)
                # v1 = E2x*px + E2y*py + E2z*pz
                nc.tensor.matmul(out=pvb[:, h, :], lhsT=wv[:, 2:4, :],
                                 rhs=t[:, 0:2, :], start=True, stop=False,
                                 perf_mode=PM.DoubleRow)
                nc.tensor.matmul(out=pvb[:, h, :], lhsT=wv[:, 4, :],
                                 rhs=t[:, 2, :], start=False, stop=True)

            # ---- PE p-state warm-up: many TINY junk matmuls (N=64) on a
            # zeroed tile while the first point chunk is in flight. The HAM
            # clock-gate needs ~3.4us of sustained PE activity to unthrottle;
            # fine-grained junk MMs cover that window yet release the PE
            # within ~60ns of the first real matmul's data arriving. ----
            wpv = pa.tile([128, 2, H], F32, name="pva", tag="pva")
            for k in range(56):
                nc.tensor.matmul(out=wpv[:, 0, 0:64], lhsT=cw[:, 0:128],
                                 rhs=cw[:, 0:64], start=True, stop=True)

            # ---- main loop ----
            # ACT: sq1 for normal groups + all sqrts; DVE: fused v0^2+sq1
            # (and sq1 via NR for HEAVY groups). Sqrt for pair p is emitted
            # one group late so the ACT FIFO never blocks a sq1.
            tots = {}
            sq7 = None

            def emit_sq1_act(g, pvb):
                sq1 = qp.tile([128, 1024], BF16, name="sq1", tag="sq1")
                nc.scalar.activation(
                    out=sq1[:, :],
                    in_=pvb[:, :, :].rearrange("p h n -> p (h n)"),
                    func=AF.Square)
                return sq1

            def emit_sq1_dve(g, pvb):
                sq1 = qp.tile([128, 1024], BF16, name="sq1", tag="sq1")
                nc.vector._custom_dve(
                    RECIPROCAL_APPROX_NR, out=sq1[:, :], in0=mo[:, :],
                    in1=pvb[:, :, :].rearrange("p h n -> p (h n)"), s0=0.0)
                return sq1

            def emit_fused(g, pva, sq1):
                if g % 2 == 0:
                    tots[g // 2] = qp.tile([128, 2, 1024], BF16,
                                           name="tot", tag="tot")
                tot = tots[g // 2]
                nc.vector._custom_dve(
                    SQ_ADD, out=tot[:, g % 2, :],
                    in0=pva[:, :, :].rearrange("p h n -> p (h n)"),
                    in1=sq1[:, :])

            def emit_sqrt(pair, slot, width=2048):
                dist = qp.tile([128, 2048], BF16, name="dist", tag="dist")
                nc.scalar.activation(
                    out=dist[:, 0:width],
                    in_=tots[pair][:, :, :].rearrange(
                        "p a b -> p (a b)")[:, 0:width],
                    func=AF.Sqrt, scale=4.0,
                    accum_out=acc[:, slot:slot + 1])

            for g in range(7):
                pva = pa.tile([128, 2, H], F32, name="pva", tag="pva")
                pvb = pb.tile([128, 2, H], F32, name="pvb", tag="pvb")
                emit_mm(pva, pvb, g, 0)
                emit_mm(pva, pvb, g, 1)
                if g in HEAVY:
                    sq1 = emit_sq1_dve(g, pvb)
                else:
                    sq1 = emit_sq1_act(g, pvb)
                # sqrt for pair p emitted two groups late so the ACT FIFO
                # never blocks a square behind a not-yet-ready tot
                if g == 2:
                    emit_sqrt(0, slot=0)
                elif g == 4:
                    emit_sqrt(1, slot=1)
                elif g == 6:
                    emit_sqrt(2, slot=2)
                emit_fused(g, pva, sq1)
                if g == 4:
                    nc.sync.dma_start(out=out[:, 0:2], in_=acc[:, 0:2])

            # group 7, per half-row-block to shorten the serial tail
            pva = pa.tile([128, 2, H], F32, name="pva", tag="pva")
            pvb = pb.tile([128, 2, H], F32, name="pvb", tag="pvb")
            sq7 = qp.tile([128, 2, H], BF16, name="sq7", tag="sq7")
            tot7 = qp.tile([128, 2, H], BF16, name="tot7", tag="tot7")
            dist7 = qp.tile([128, 2, H], BF16, name="dist7", tag="dist7")
            dist6 = qp.tile([128, 1024], BF16, name="dist6", tag="dist6")
            def sq_add_7(h):
                nc.vector._custom_dve(
                    SQ_ADD, out=tot7[:, h, :], in0=pva[:, h, :],
                    in1=sq7[:, h, :])

            def sqrt_7(h):
                # per-half sqrt keeps the serial tail short: only h1's
                # 512-wide chain runs after the last matmul
                nc.scalar.activation(
                    out=dist7[:, h, :], in_=tot7[:, h, :],
                    func=AF.Sqrt, scale=4.0, accum_out=acc[:, 4 + h:5 + h])

            emit_mm(pva, pvb, 7, 0)
            nc.scalar.activation(out=sq7[:, 0, :], in_=pvb[:, 0, :],
                                 func=AF.Square)
            sq_add_7(0)
            # sqrt for group 6 rides the h1-matmul window
            nc.scalar.activation(
                out=dist6[:, :], in_=tots[3][:, 0, :],
                func=AF.Sqrt, scale=4.0, accum_out=acc[:, 3:4])
            nc.sync.dma_start(out=out[:, 2:4], in_=acc[:, 2:4])
            emit_mm(pva, pvb, 7, 1)
            nc.scalar.activation(out=sq7[:, 1, :], in_=pvb[:, 1, :],
                                 func=AF.Square)
            sq_add_7(1)
            sqrt_7(0)
            sqrt_7(1)
            nc.sync.dma_start(out=out[:, 4:6], in_=acc[:, 4:6])

    nc.finalize()
    return nc


# ---------------- host-side coefficient math ----------------

def _host_ew(pred, mode, gt):
    """E1/E2 per batch row, float64, mirroring the reference math."""
    p = pred.astype(np.float64)
    md = mode.astype(np.float64)
    m1, m2, m3, m4 = p[:, 0], p[:, 1], p[:, 2], p[:, 3]
    sgn = np.where(md > 0.5, 1.0, -1.0)
    e2 = sgn * np.arcsin(np.sqrt(m3 ** 2 / (m1 ** 2 + m2 ** 2 + m3 ** 2)))
    e3 = np.arctan2(m4, m3 / (np.sin(e2) + 1e-9))
    tmp = np.cos(e2) * np.cos(e3)
    e1 = np.arctan2(m2 / tmp, m1 / tmp)
    e3 = np.where(e3 > 0, e3, e3 + 2 * np.pi)
    ep = np.stack([e1, e2, e3], -1)
    eg = gt.astype(np.float64)

    def quat_xyz(e):
        # q = qx(a) * qy(b) * qz(c) for R = Rx(a) Ry(b) Rz(c)
        a, b, c = e[:, 0] / 2, e[:, 1] / 2, e[:, 2] / 2
        ca, sa = np.cos(a), np.sin(a)
        cb, sb = np.cos(b), np.sin(b)
        cc, sc = np.cos(c), np.sin(c)
        w = ca * cb * cc - sa * sb * sc
        x = sa * cb * cc + ca * sb * sc
        y = ca * sb * cc - sa * cb * sc
        z = ca * cb * sc + sa * sb * cc
        return w, np.stack([x, y, z], -1)

    wp, vp = quat_xyz(ep)
    wg, vg = quat_xyz(eg)
    qv = wg[:, None] * vp - wp[:, None] * vg - np.cross(vp, vg)

    qx, qy, qz = qv[:, 0], qv[:, 1], qv[:, 2]
    s = qy ** 2 + qz ** 2
    n = np.sqrt(s + qx ** 2)
    r = 1.0 / np.sqrt(s + 1e-250)
    t1 = n * r
    # E1 = (0, qz, -qy) * t1  (x component identically zero -> dropped)
    # E2 = (-s, qx*qy, qx*qz) * r
    return np.stack([qz * t1, -qy * t1,
                     -s * r, qx * qy * r, qx * qz * r], -1)   # [B, 5]


def _pack_inputs(pred, mode, gt, point):
    w5 = _host_ew(np.asarray(pred), np.asarray(mode), np.asarray(gt))
    w5q = w5.astype(np.float32).astype(E4M3)           # [B, 5]
    ptq = np.asarray(point, dtype=np.float32).astype(E4M3)  # [B, N, 3]

    in_maps = []
    idx = np.arange(128)
    for c in range(NCORES):
        sl = slice(c * BSH, (c + 1) * BSH)
        # row b_local = p*G + g
        w5c = w5q[sl].reshape(128, G, 5)
        wtc = np.zeros((128, G, 5, 128), dtype=E4M3)
        wtc[idx, :, :, idx] = w5c
        ptc = (ptq[sl].reshape(128, G, 2, H, 3)
               .transpose(0, 1, 2, 4, 3))              # [p, g, h, c, n]
        chunk = np.concatenate(
            [wtc.reshape(128, G, W_G), ptc.reshape(128, G, 2 * PT_H)], axis=2)
        in_maps.append({"ptw": np.ascontiguousarray(chunk)
                        .reshape(128, G * C_G)})
    return in_maps


def _get_nc():
    if "nc" not in _CACHE:
        _CACHE["nc"] = build_bass()
    return _CACHE["nc"]


def kernel(pred, mode, gt, point, **run_kwargs):
    nc = _get_nc()
    in_maps = _pack_inputs(pred, mode, gt, point)
    res = run_bass_kernel_spmd(nc, in_maps, core_ids=list(range(NCORES)),
                               **run_kwargs)
    total = sum(float(r["out"][:, 0:6].astype(np.float64).sum())
                for r in res.results)
    result = np.float32(total / (B * N))
    if run_kwargs:
        return result, res
    return result
